# revision 9
# baseline (speedup 1.0000x reference)
"""Trainium2 Bass kernel for nn_MultiHeadCovProbeV2.

Data-parallel over batch B=8: core i processes batch i. Params are
replicated to every core. Self-contained: shapes hardcoded.

Per-core pipeline (S=2048, D=4096, H=64):
  x [S, D] --DMA(cast f32->f32r)--> SBUF natural tiles [128, D]
    --PE transpose (f32r)--> PSUM --DVE copy--> xT "quad" tiles [128 d, 4*SB]
  combined projection (Wl|Wr stacked): psum[128, SB] += WT_k^T @ xT_k
    (f32r matmuls, moving free dim SB=256 -> full PE rate)
  +bias, PE back-transpose to [s, (l|r)], *mask, cov[64,64] PSUM accumulation
  cov/len + eps*I -> Newton-Schulz sqrtm (3 iters, 64x64 fp32 matmuls,
    transpose-tracked to avoid PE transposes in the chain)
  factored bilinear heads + output projections -> out [1, 111]
"""
import sys

for p in ("/opt/trn_rl_repo", "/root/.axon_site/_ro/trn_rl_repo"):
    if p not in sys.path:
        sys.path.append(p)

import numpy as np
import concourse.bass as bass
import concourse.mybir as mybir
from concourse.tile import TileContext
from concourse.masks import make_identity

F32 = mybir.dt.float32
F32R = mybir.dt.float32r
U8 = mybir.dt.uint8
ALU = mybir.AluOpType

B = 8
S = 2048
D = 4096
H = 64          # d_hidden
DP = 128        # d_probe
HEADS = (10, 100, 1)
HTOT = sum(HEADS)
EPS = 1e-3
NITER = 3
N_CORES = 8
ACT_COPY_QUADS = False
ACT_MASK_MUL = False


def _split_multi_waits(nc):
    """walrus in this toolchain encodes at most one sem-wait per
    instruction; Tile's exit drain (and a few scheduled instructions) carry
    several. Split extras onto single-wait NOPs preceding the instruction."""
    n = 0
    for f in nc.m.functions:
        for bb in f.blocks:
            out = []
            changed = False
            for inst in bb.instructions:
                si = inst.sync_info
                if si is not None and si.on_wait and len(si.on_wait) > 1:
                    waits = list(si.on_wait)
                    for w in waits[:-1]:
                        n += 1
                        nop = mybir.InstNoOp(name=f"I-wsplit-{n}",
                                             engine=inst.engine)
                        nop.sync_info = mybir.SyncInfo(on_wait=[w],
                                                       on_update=[])
                        out.append(nop)
                    si.on_wait = [waits[-1]]
                    changed = True
                out.append(inst)
            if changed:
                bb.instructions = out
    return n


def build_program(use_f32r=True, split=True, reps=1):
    P = 128
    NT = S // P                  # 16 s-tiles
    SB = 256                     # s-block (proj moving free dim)
    NSB = S // SB                # 8
    TPB = SB // P                # 2 s-tiles per block
    DCH = D // 128               # 32 d chunks
    NQ = DCH // 4                # 8 quad groups

    dt_x = F32R if use_f32r else F32

    nc = bass.Bass()
    x = nc.dram_tensor("x", [S, D], F32, kind="ExternalInput")
    mask = nc.dram_tensor("mask", [S], U8, kind="ExternalInput")
    wl = nc.dram_tensor("wl", [H, D], F32, kind="ExternalInput")
    wr = nc.dram_tensor("wr", [H, D], F32, kind="ExternalInput")
    blb = nc.dram_tensor("blb", [H], F32, kind="ExternalInput")
    brb = nc.dram_tensor("brb", [H], F32, kind="ExternalInput")
    hlw = nc.dram_tensor("hlw", [3, DP, H], F32, kind="ExternalInput")
    hrw = nc.dram_tensor("hrw", [3, DP, H], F32, kind="ExternalInput")
    w0 = nc.dram_tensor("w0", [HEADS[0], DP], F32, kind="ExternalInput")
    w1 = nc.dram_tensor("w1", [HEADS[1], DP], F32, kind="ExternalInput")
    w2 = nc.dram_tensor("w2", [HEADS[2], DP], F32, kind="ExternalInput")
    b0 = nc.dram_tensor("b0", [HEADS[0]], F32, kind="ExternalInput")
    b1 = nc.dram_tensor("b1", [HEADS[1]], F32, kind="ExternalInput")
    b2 = nc.dram_tensor("b2", [HEADS[2]], F32, kind="ExternalInput")
    out = nc.dram_tensor("out", [1, HTOT], F32, kind="ExternalOutput")

    with TileContext(nc) as tc, \
         tc.tile_pool(name="const", bufs=1) as const, \
         tc.tile_pool(name="xa", bufs=2) as xa_pool, \
         tc.tile_pool(name="quads", bufs=1) as quad_pool, \
         tc.tile_pool(name="lr", bufs=2) as lr_pool, \
         tc.tile_pool(name="lrTs", bufs=2) as lrT_pool, \
         tc.tile_pool(name="ns", bufs=1) as ns_pool, \
         tc.tile_pool(name="tp", bufs=4, space="PSUM") as tp_ps, \
         tc.tile_pool(name="pj", bufs=2, space="PSUM") as pj_ps, \
         tc.tile_pool(name="sm", bufs=1, space="PSUM") as sm_ps, \
         tc.tile_pool(name="cv", bufs=1, space="PSUM") as cov_ps:

        # ---- constants / setup ----
        ident_f = const.tile([128, 128], F32)
        make_identity(nc, ident_f[:, :])
        if use_f32r:
            ident_r = const.tile([128, 128], F32R)
            nc.vector.tensor_copy(ident_r[:, :], ident_f[:, :])
        else:
            ident_r = ident_f
        ones_col = const.tile([128, 1], F32)
        nc.vector.memset(ones_col[:, :], 1.0)
        ones_row = const.tile([1, 128], F32)
        nc.vector.memset(ones_row[:, :], 1.0)
        i15 = const.tile([H, H], F32)
        nc.vector.tensor_scalar_mul(i15[:, :], ident_f[0:H, 0:H], 1.5)
        epsI = const.tile([H, H], F32)
        nc.vector.tensor_scalar_mul(epsI[:, :], ident_f[0:H, 0:H], EPS)

        act_warm = const.tile([1, 1], F32)
        nc.scalar.activation(act_warm[:, :], ones_col[0:1, 0:1],
                             mybir.ActivationFunctionType.Sqrt)

        def bcast_col(name, val_ap, p=H):
            ps = sm_ps.tile([128, 512], F32, name=f"pb_{name}", tag="sm")
            nc.tensor.matmul(ps[0:p, 0:1], ones_row[0:1, 0:p], val_ap,
                             start=True, stop=True)
            col = const.tile([p, 1], F32, name=f"bc_{name}", tag=f"bc_{name}")
            nc.vector.tensor_copy(col[:, :], ps[0:p, 0:1])
            return col

        # combined projection weights -> transposed chunks [128 d, 128 (l|r)]
        w_nat = const.tile([128, D], F32)
        dma_cast = nc.gpsimd.dma_start if use_f32r else nc.sync.dma_start
        nc.sync.dma_start(out=w_nat[0:H, :], in_=wl[:, :])
        nc.sync.dma_start(out=w_nat[H:2 * H, :], in_=wr[:, :])
        wT = []
        for k in range(DCH):
            pw = tp_ps.tile([128, 512], F32, name=f"pw{k}", tag="tp")
            nc.tensor.transpose(pw[:, 0:128], w_nat[:, k * 128:(k + 1) * 128],
                                ident_f[:, :])
            wt_k = const.tile([128, 128], dt_x, name=f"wT{k}", tag=f"wT{k}")
            nc.vector.tensor_copy(wt_k[:, :], pw[:, 0:128])
            wT.append(wt_k)

        bias_col = const.tile([128, 1], F32)
        nc.sync.dma_start(out=bias_col[0:H, 0:1],
                          in_=blb[:].rearrange("(h o) -> h o", o=1))
        nc.sync.dma_start(out=bias_col[H:2 * H, 0:1],
                          in_=brb[:].rearrange("(h o) -> h o", o=1))

        # mask [S] u8 -> mask_cols [128, NT] f32 (col t = s-tile t)
        m_u8 = const.tile([NT, 128], U8)
        nc.sync.dma_start(out=m_u8[:, :],
                          in_=mask[:].rearrange("(t p) -> t p", p=128))
        m_f = const.tile([NT, 128], F32)
        nc.vector.tensor_copy(m_f[:, :], m_u8[:, :])
        pm = sm_ps.tile([128, 512], F32, name="pm", tag="sm")
        nc.tensor.transpose(pm[:, 0:NT], m_f[:, :], ident_f[0:NT, 0:NT])
        mask_cols = const.tile([128, NT], F32)
        nc.vector.tensor_copy(mask_cols[:, :], pm[:, 0:NT])
        msum = const.tile([128, 1], F32)
        nc.vector.reduce_sum(msum[:, :], mask_cols[:, :],
                             axis=mybir.AxisListType.X)
        pL = sm_ps.tile([1, 512], F32, name="pL", tag="sm")
        nc.tensor.matmul(pL[0:1, 0:1], msum[:, :], ones_col[:, :], start=True,
                         stop=True)
        recipL = const.tile([1, 1], F32)
        nc.vector.tensor_scalar_max(recipL[:, :], pL[0:1, 0:1], 1.0)
        nc.vector.reciprocal(recipL[:, :], recipL[:, :])
        recipL_h = bcast_col("recipL", recipL[0:1, 0:1])

        # factored head weights
        hr_nat, hlT = [], []
        for n in range(3):
            hn = const.tile([DP, H], F32, name=f"hl{n}", tag=f"hl{n}")
            nc.sync.dma_start(out=hn[:, :], in_=hlw[n, :, :])
            rn = const.tile([DP, H], F32, name=f"hr{n}", tag=f"hr{n}")
            nc.sync.dma_start(out=rn[:, :], in_=hrw[n, :, :])
            hr_nat.append(rn)
            ph = tp_ps.tile([128, 512], F32, name=f"ph{n}", tag="tp")
            nc.tensor.transpose(ph[0:H, 0:DP], hn[:, :], ident_f[:, :])
            ht = const.tile([H, DP], F32, name=f"hlT{n}", tag=f"hlT{n}")
            nc.vector.tensor_copy(ht[:, :], ph[0:H, 0:DP])
            hlT.append(ht)

        # output head weights transposed [128, hs]
        woT = []
        for n, (wn, hs) in enumerate(zip((w0, w1, w2), HEADS)):
            wn_nat = const.tile([hs, DP], F32, name=f"wo{n}", tag=f"wo{n}")
            nc.sync.dma_start(out=wn_nat[:, :], in_=wn[:, :])
            pw = tp_ps.tile([128, 512], F32, name=f"pwo{n}", tag="tp")
            nc.tensor.transpose(pw[:, 0:hs], wn_nat[:, :],
                                ident_f[0:hs, 0:hs])
            wt = const.tile([DP, hs], F32, name=f"woT{n}", tag=f"woT{n}")
            nc.vector.tensor_copy(wt[:, :], pw[:, 0:hs])
            woT.append(wt)
        bias_row = const.tile([1, HTOT], F32)
        off = 0
        for bn, hs in zip((b0, b1, b2), HEADS):
            nc.sync.dma_start(out=bias_row[0:1, off:off + hs],
                              in_=bn[:].rearrange("(o h) -> o h", o=1))
            off += hs

        # ---- main pipeline (optionally repeated for marginal timing) ----
        for rep in range(reps):
            rs = f"r{rep}_" if reps > 1 else ""
            cov = cov_ps.tile([H, H], F32, name=f"{rs}cov", tag="cv")
            quads = [quad_pool.tile([128, 4 * SB], dt_x, name=f"{rs}q{q}",
                                    tag=f"q{q}") for q in range(NQ)]

            for blk in range(NSB):
                xab = xa_pool.tile([128, TPB * D], dt_x, name=f"{rs}xab{blk}",
                                   tag="xa")
                src3 = x[:, :].rearrange("(n p) d -> p n d", p=128)[
                    :, blk * TPB:(blk + 1) * TPB, :]
                dma_cast(out=xab[:, :].rearrange("p (j d) -> p j d", j=TPB),
                         in_=src3)
                for j in range(TPB):
                    t = blk * TPB + j
                    xa = xab[:, j * D:(j + 1) * D]
                    for q in range(NQ):
                        pt = tp_ps.tile([128, 512], dt_x,
                                        name=f"{rs}pt{t}_{q}", tag="tp")
                        for i in range(4):
                            k = 4 * q + i
                            nc.tensor.transpose(pt[:, i * 128:(i + 1) * 128],
                                                xa[:, k * 128:(k + 1) * 128],
                                                ident_r[:, :])
                        # chunk c of quad q -> cols [c*SB + j*128, +128)
                        dst = quads[q].rearrange(
                            "p (c s) -> p c s", c=4)[:, :,
                                                     j * 128:(j + 1) * 128]
                        srcap = pt[:, :].rearrange("p (c s) -> p c s", c=4)
                        srcap = srcap.bitcast(F32) if use_f32r else srcap
                        if ACT_COPY_QUADS and q in (1, 4, 6):
                            nc.scalar.copy(dst, srcap)
                        else:
                            nc.vector.tensor_copy(dst, srcap)

                # combined projection for this s-block
                pp = pj_ps.tile([128, SB], F32, name=f"{rs}pp{blk}", tag="pj")
                for k in range(DCH):
                    q, i = k // 4, k % 4
                    nc.tensor.matmul(pp[:, :], wT[k][:, :],
                                     quads[q][:, i * SB:(i + 1) * SB],
                                     start=(k == 0), stop=(k == DCH - 1))
                lr = lr_pool.tile([128, SB], F32, name=f"{rs}lr{blk}",
                                  tag="lr")
                nc.vector.tensor_scalar_add(lr[:, :], pp[:, :],
                                            bias_col[:, 0:1])

                # back-transpose, mask, cov accumulation
                for j in range(TPB):
                    t = blk * TPB + j
                    ptl = sm_ps.tile([128, 512], F32, name=f"{rs}ptl{t}",
                                     tag="sm")
                    nc.tensor.transpose(ptl[:, 0:128],
                                        lr[:, j * 128:(j + 1) * 128],
                                        ident_f[:, :])
                    lrT = lrT_pool.tile([128, 128], F32, name=f"{rs}lrT{t}",
                                        tag="lrTs")
                    if ACT_MASK_MUL:
                        nc.scalar.mul(lrT[:, :], ptl[:, 0:128],
                                      mask_cols[:, t:t + 1])
                    else:
                        nc.vector.tensor_scalar_mul(lrT[:, :], ptl[:, 0:128],
                                                    mask_cols[:, t:t + 1])
                    nc.tensor.matmul(cov[:, :], lrT[:, 0:H], lrT[:, H:2 * H],
                                     start=(t == 0), stop=(t == NT - 1))

            # ---- A = cov/L + eps*I ----
            A = ns_pool.tile([H, H], F32, name=f"{rs}A", tag="A")
            nc.vector.scalar_tensor_tensor(A[:, :], cov[:, :],
                                           recipL_h[:, 0:1], epsI[:, :],
                                           op0=ALU.mult, op1=ALU.add)

            # ---- Frobenius norm ----
            scr = ns_pool.tile([H, H], F32, name=f"{rs}scr", tag="scr")
            sq_col = ns_pool.tile([H, 1], F32, name=f"{rs}sq_col",
                                  tag="sq_col")
            nc.vector.tensor_tensor(out=scr[:, :], in0=A[:, :], in1=A[:, :],
                                    op=ALU.mult)
            nc.vector.reduce_sum(sq_col[:, :], scr[:, :],
                                 axis=mybir.AxisListType.X)
            pS = sm_ps.tile([1, 512], F32, name=f"{rs}pS", tag="sm")
            nc.tensor.matmul(pS[0:1, 0:1], sq_col[:, :], ones_col[0:H, :],
                             start=True, stop=True)
            normA = ns_pool.tile([1, 1], F32, name=f"{rs}normA", tag="normA")
            nc.scalar.activation(normA[:, :], pS[0:1, 0:1],
                                 mybir.ActivationFunctionType.Sqrt)
            snorm = ns_pool.tile([1, 1], F32, name=f"{rs}snorm", tag="snorm")
            nc.scalar.activation(snorm[:, :], normA[:, :],
                                 mybir.ActivationFunctionType.Sqrt)
            rnorm = ns_pool.tile([1, 1], F32, name=f"{rs}rnorm", tag="rnorm")
            nc.vector.reciprocal(rnorm[:, :], normA[:, :])
            rnorm_h = bcast_col(f"{rs}rnorm", rnorm[0:1, 0:1])
            snorm_h = bcast_col(f"{rs}snorm", snorm[0:1, 0:1])

            # ---- Newton-Schulz (transpose-tracked) ----
            def mm(name, lhsT, rhs, m=H, n=H):
                ps = pj_ps.tile([128, SB], F32, name=f"{rs}ps_{name}",
                                tag="pj")
                nc.tensor.matmul(ps[0:m, 0:n], lhsT, rhs, start=True,
                                 stop=True)
                return ps[0:m, 0:n]

            def to_sb(name, ps_ap):
                sb = ns_pool.tile([H, H], F32, name=f"{rs}{name}", tag=name)
                nc.vector.tensor_copy(sb[:, :], ps_ap)
                return sb

            Y = ns_pool.tile([H, H], F32, name=f"{rs}Y", tag="Y")
            nc.vector.tensor_scalar_mul(Y[:, :], A[:, :], rnorm_h[:, 0:1])
            Yt = to_sb("Yt0", mm("y0t", Y[:, :], ident_f[0:H, 0:H]))
            Z = ident_f[0:H, 0:H]
            Zt = ident_f[0:H, 0:H]

            for it in range(NITER):
                W_ps = mm(f"W{it}", Zt[:, :], Y[:, :])
                T = ns_pool.tile([H, H], F32, name=f"{rs}T{it}", tag="Tns")
                nc.vector.scalar_tensor_tensor(T[:, :], W_ps, -0.5, i15[:, :],
                                               op0=ALU.mult, op1=ALU.add)
                if it < NITER - 1:
                    Wt_ps = mm(f"Wt{it}", Y[:, :], Zt[:, :])
                    Tt = ns_pool.tile([H, H], F32, name=f"{rs}Tt{it}",
                                      tag="Ttns")
                    nc.vector.scalar_tensor_tensor(Tt[:, :], Wt_ps, -0.5,
                                                   i15[:, :], op0=ALU.mult,
                                                   op1=ALU.add)
                    Yn = to_sb(f"Y{it + 1}", mm(f"mYn{it}", Yt[:, :],
                                                T[:, :]))
                    Ytn = to_sb(f"Yt{it + 1}", mm(f"mYtn{it}", T[:, :],
                                                  Yt[:, :]))
                    Ztn = to_sb(f"Zt{it + 1}", mm(f"mZtn{it}", Z[:, :],
                                                  Tt[:, :]))
                    if it < NITER - 2:
                        Zn = to_sb(f"Z{it + 1}", mm(f"mZn{it}", Tt[:, :],
                                                    Z[:, :]))
                    else:
                        Zn = None
                    Y, Yt, Z, Zt = Yn, Ytn, Zn, Ztn
                else:
                    Y = to_sb(f"Y{it + 1}", mm(f"mYn{it}", Yt[:, :], T[:, :]))

            Ys = ns_pool.tile([H, H], F32, name=f"{rs}Ys", tag="Ys")
            nc.vector.tensor_scalar_mul(Ys[:, :], Y[:, :], snorm_h[:, 0:1])

            # ---- heads ----
            out_ps = sm_ps.tile([1, 512], F32, name=f"{rs}out_ps", tag="sm")
            off = 0
            for n, hs in enumerate(HEADS):
                Mh = mm(f"Mh{n}", hlT[n][:, :], Ys[:, :], m=DP, n=H)
                scrM = ns_pool.tile([DP, H], F32, name=f"{rs}scrM{n}",
                                    tag="scrM")
                hid = ns_pool.tile([DP, 1], F32, name=f"{rs}hid{n}",
                                   tag=f"hid{n}")
                nc.vector.tensor_tensor(out=scrM[:, :], in0=Mh,
                                        in1=hr_nat[n][:, :], op=ALU.mult)
                nc.vector.reduce_sum(hid[:, :], scrM[:, :],
                                     axis=mybir.AxisListType.X)
                nc.tensor.matmul(out_ps[0:1, off:off + hs], hid[:, :],
                                 woT[n][:, :], start=(n == 0), stop=(n == 2))
                off += hs
            out_sb = ns_pool.tile([1, HTOT], F32, name=f"{rs}out_sb",
                                  tag="out_sb")
            nc.vector.tensor_tensor(out=out_sb[:, :], in0=out_ps[0:1, 0:HTOT],
                                    in1=bias_row[:, :], op=ALU.add)
            nc.sync.dma_start(out=out[:, :], in_=out_sb[:, :])

    if split:
        _split_multi_waits(nc)
    return nc


_nc_cache = {}


def get_program():
    if "nc" not in _nc_cache:
        _nc_cache["nc"] = build_program(use_f32r=True)
    return _nc_cache["nc"]


def make_in_maps(inputs):
    x = np.ascontiguousarray(inputs["x"], dtype=np.float32)
    mask = np.asarray(inputs["attn_mask"])
    mask_u8 = np.ascontiguousarray(mask.astype(np.uint8))
    shared = {
        "wl": np.ascontiguousarray(inputs["proj_left_w"], np.float32),
        "wr": np.ascontiguousarray(inputs["proj_right_w"], np.float32),
        "blb": np.ascontiguousarray(inputs["proj_left_b"], np.float32),
        "brb": np.ascontiguousarray(inputs["proj_right_b"], np.float32),
        "hlw": np.ascontiguousarray(inputs["head_left"], np.float32),
        "hrw": np.ascontiguousarray(inputs["head_right"], np.float32),
        "w0": np.ascontiguousarray(inputs["out_w0"], np.float32),
        "w1": np.ascontiguousarray(inputs["out_w1"], np.float32),
        "w2": np.ascontiguousarray(inputs["out_w2"], np.float32),
        "b0": np.ascontiguousarray(inputs["out_b0"], np.float32),
        "b1": np.ascontiguousarray(inputs["out_b1"], np.float32),
        "b2": np.ascontiguousarray(inputs["out_b2"], np.float32),
    }
    return [dict(shared, x=x[i], mask=mask_u8[i]) for i in range(B)]


def kernel(**inputs) -> np.ndarray:
    from concourse.bass_utils import run_bass_kernel_spmd

    nc = get_program()
    in_maps = make_in_maps(inputs)
    res = run_bass_kernel_spmd(nc, in_maps, list(range(N_CORES)))
    return np.concatenate([res.results[i]["out"] for i in range(B)],
                          axis=0).astype(np.float32)
